# revision 7
# baseline (speedup 1.0000x reference)
"""Trainium2 Bass kernel for nn_Conv2D_ConvLSTM1D (Conv2D stack -> 2x ConvLSTM1D -> FC).

Sharding: data-parallel over batch. 64 batches / 8 cores = 8 per core.
Each core runs the identical program on its batch shard; no collectives.

Per-core layout: channels on SBUF partitions, (batch, station) on the free
dim.  All convolutions become PE matmuls: contract dim = input channels
(or 3x3 taps for conv1), shifts over stations/time handled by sliding AP
windows into zero-padded SBUF tiles.  The whole network runs per-time-step
so every intermediate stays SBUF-resident (no DRAM spill).

Engine assignment (v1 restructure):
  - hard-sigmoid gates: ACT Relu(0.2*psum + (0.2*b+0.5)) from PSUM; the
    min(.,1) upper clamp is folded into the consuming multiply via DVE
    scalar_tensor_tensor (out = (gate min 1) * other).  No GpSimd in the
    loop (its tensor ops run ~25x slower than DVE).
  - LSTM1 x-conv and h-conv fused: [h1 | hn] stacked on 128 partitions
    with merged [kh1; kx1] weights -> 3 matmuls per (group, pair).
  - L2-norm square on ACT (Square); reduce + reciprocal on DVE; Sqrt on
    ACT (only cross-table activation, 2 table loads/step).

Matmul operands are bf16 (fp32 PSUM accumulation, fp32 cell state and gate
math).
"""
import sys

if "/opt/trn_rl_repo" not in sys.path:
    sys.path.insert(0, "/opt/trn_rl_repo")

import numpy as np

B_FULL, T, N, CIN = 64, 24, 256, 1
F1, F2 = 64, 128
NCORES = 8
BL = B_FULL // NCORES          # batch per core
PAIRS = BL // 2                # matmuls process 2 batches (512 cols) at once

_CACHE = {}


def _legalize_waits(nc, mybir):
    """TRN2 engine instructions carry at most 1 sync wait (EventSemaphore: 2).
    Tile emits more; split extras onto injected NoOps placed just before."""
    n = 0
    for _, bbobj in list(nc.bb_map.items()):
        bb = bbobj.bb if hasattr(bbobj, "bb") else bbobj
        insts = bb.instructions
        newlist = []
        for inst in insts:
            si = getattr(inst, "sync_info", None)
            waits = list(si.on_wait) if (si is not None and si.on_wait) else []
            cap = 2 if isinstance(inst, mybir.InstEventSemaphore) else 1
            if len(waits) > cap:
                extra, keep = waits[:-cap], waits[-cap:]
                for w in extra:
                    n += 1
                    nop = mybir.InstNoOp(name=f"I-wf{n}", ins=[], outs=[])
                    nop.engine = inst.engine
                    nop.sync_info = mybir.SyncInfo(on_wait=[w], on_update=[])
                    nc.inst_map[nop.name] = nop
                    newlist.append(nop)
                inst.sync_info = mybir.SyncInfo(
                    on_wait=keep,
                    on_update=list(si.on_update) if si.on_update else [])
            newlist.append(inst)
        insts[:] = newlist
    return n


def _build_nc():
    import concourse.bass as bass
    import concourse.tile as tile
    from concourse import mybir
    from concourse.alu_op_type import AluOpType as Op

    F32 = mybir.dt.float32
    BF16 = mybir.dt.bfloat16
    AF = mybir.ActivationFunctionType
    AX = mybir.AxisListType

    nc = bass.Bass()

    x_d = nc.dram_tensor("x", [BL, T, N, CIN], F32, kind="ExternalInput")
    c1w_d = nc.dram_tensor("conv1_w", [3, 3, CIN, F1], F32, kind="ExternalInput")
    c1b_d = nc.dram_tensor("conv1_b", [F1], F32, kind="ExternalInput")
    c2w_d = nc.dram_tensor("conv2_w", [3, 3, F1, F1], F32, kind="ExternalInput")
    c2b_d = nc.dram_tensor("conv2_b", [F1], F32, kind="ExternalInput")
    kx1_d = nc.dram_tensor("kx1", [3, F1, 4 * F1], F32, kind="ExternalInput")
    kh1_d = nc.dram_tensor("kh1", [3, F1, 4 * F1], F32, kind="ExternalInput")
    b1_d = nc.dram_tensor("b1", [4 * F1], F32, kind="ExternalInput")
    kx2_d = nc.dram_tensor("kx2", [3, F1, 4 * F2], F32, kind="ExternalInput")
    kh2_d = nc.dram_tensor("kh2", [3, F2, 4 * F2], F32, kind="ExternalInput")
    b2_d = nc.dram_tensor("b2", [4 * F2], F32, kind="ExternalInput")
    fcw_d = nc.dram_tensor("fc_w", [F2, 1], F32, kind="ExternalInput")
    fcb_d = nc.dram_tensor("fc_b", [1], F32, kind="ExternalInput")
    y_d = nc.dram_tensor("y", [BL, N], F32, kind="ExternalOutput")

    xd = x_d.rearrange("b t n c -> b t (n c)")  # [BL, T, N]

    # L1 gate order remap: groups [f|i], [o|gc]; source order is [i|f|gc|o]
    L1_SRC = [1, 0, 3, 2]  # dest block j <- source block L1_SRC[j]

    with tile.TileContext(nc) as tc:
        with (
            tc.tile_pool(name="wp", bufs=1) as wp,
            tc.tile_pool(name="st", bufs=1) as st,
            tc.tile_pool(name="io", bufs=2) as io,
            tc.tile_pool(name="gt", bufs=1) as gt,
            tc.tile_pool(name="sc", bufs=2) as scp,
            tc.tile_pool(name="pp", bufs=6, space="PSUM") as pp,
            tc.tile_pool(name="pS", bufs=2, space="PSUM") as pS,
        ):
            # ---------------- setup: weights (bf16 via casting gpsimd DMA) ----
            w1 = wp.tile([9, F1], BF16)
            nc.gpsimd.dma_start(w1[:], c1w_d.rearrange("a b c d -> (a b c) d"))
            w2 = wp.tile([F1, 9, F1], BF16)
            nc.gpsimd.dma_start(w2[:], c2w_d.rearrange("a b c d -> c (a b) d"))
            # merged LSTM1 weights: rows 0:64 <- kh1 (h1 part),
            # rows 64:128 <- kx1 (hn part); gate-block order [f|i|o|gc]
            kxh1 = wp.tile([128, 3, 4 * F1], BF16)
            kh1r = kh1_d.rearrange("k c f -> c k f")
            kx1r = kx1_d.rearrange("k c f -> c k f")
            for j, s in enumerate(L1_SRC):
                nc.gpsimd.dma_start(kxh1[0:F1, :, 64 * j:64 * j + 64],
                                    kh1r[:, :, 64 * s:64 * s + 64])
                nc.gpsimd.dma_start(kxh1[F1:128, :, 64 * j:64 * j + 64],
                                    kx1r[:, :, 64 * s:64 * s + 64])
            kx2 = wp.tile([F1, 3, 4 * F2], BF16)
            nc.gpsimd.dma_start(kx2[:], kx2_d.rearrange("k c f -> c k f"))
            kh2 = wp.tile([F2, 3, 4 * F2], BF16)
            nc.gpsimd.dma_start(kh2[:], kh2_d.rearrange("k c f -> c k f"))
            fcw = wp.tile([F2, 1], BF16)
            nc.gpsimd.dma_start(fcw[:], fcw_d[:])

            # biases / consts (fp32)
            c1b = wp.tile([F1, 1], F32)
            nc.sync.dma_start(c1b[:], c1b_d.rearrange("(f u) -> f u", u=1))
            c2b = wp.tile([F1, 1], F32)
            nc.sync.dma_start(c2b[:], c2b_d.rearrange("(f u) -> f u", u=1))
            b1r = b1_d.rearrange("(f u) -> f u", u=1)
            b1s = wp.tile([128, 2], F32)
            for j, s in enumerate(L1_SRC):
                g, h = divmod(j, 2)   # dest: col g, partition-half h
                nc.sync.dma_start(b1s[64 * h:64 * h + 64, g:g + 1],
                                  b1r[64 * s:64 * s + 64, :])
            b2s = wp.tile([128, 4], F32)
            nc.sync.dma_start(b2s[:], b2_d.rearrange("(g p) -> p g", g=4))
            fcb = wp.tile([1, 1], F32)
            nc.sync.dma_start(fcb[:], fcb_d.rearrange("(f u) -> f u", u=1))

            # hard-sigmoid-folded additive consts:
            #   hs gates: gate = Relu(0.2*psum + (0.2*b + 0.5)), min 1 folded
            #   gc gates: gate = tanh(psum + b)
            # L1: col0 = [f|i] all hs; col1 = [o(hs) | gc(raw b)]
            addc1 = wp.tile([128, 2], F32)
            nc.vector.tensor_scalar(out=addc1[:, 0:1], in0=b1s[:, 0:1],
                                    scalar1=0.2, scalar2=0.5,
                                    op0=Op.mult, op1=Op.add)
            nc.vector.tensor_scalar(out=addc1[0:64, 1:2], in0=b1s[0:64, 1:2],
                                    scalar1=0.2, scalar2=0.5,
                                    op0=Op.mult, op1=Op.add)          # o rows
            nc.vector.tensor_copy(addc1[64:128, 1:2], b1s[64:128, 1:2])  # gc rows
            # L2: groups [i|f|gc|o]: hs cols 0,1,3; gc col2 raw
            addc2 = wp.tile([128, 4], F32)
            nc.vector.tensor_scalar(out=addc2[:], in0=b2s[:],
                                    scalar1=0.2, scalar2=0.5,
                                    op0=Op.mult, op1=Op.add)
            nc.vector.tensor_copy(addc2[:, 2:3], b2s[:, 2:3])         # gc group
            eps = wp.tile([128, 1], F32)
            nc.vector.memset(eps[:], 1e-12)
            zero = wp.tile([128, 1], F32)
            nc.vector.memset(zero[:], 0.0)
            ones = wp.tile([128, 128], F32)
            nc.vector.memset(ones[:], 1.0)

            # ---------------- persistent state ----------------
            hconv = st.tile([F1, 3, BL, N + 2], BF16)   # conv1 out, 3-slot t-window
            nc.vector.memset(hconv[:], 0.0)
            # xh1: [h1(t-1) rows 0:64 | hn(t) rows 64:128], double-buffered
            xh1 = [st.tile([128, BL, N + 2], BF16, tag=f"xh{i}", name=f"xh{i}")
                   for i in range(2)]
            for tl in xh1:
                nc.vector.memset(tl[:], 0.0)
            c1t = st.tile([F1, BL, N], F32)             # LSTM1 cell
            nc.vector.memset(c1t[:], 0.0)
            h2p = st.tile([F2, BL, N + 2], BF16)        # LSTM2 hidden (padded)
            nc.vector.memset(h2p[:], 0.0)
            c2t = st.tile([F2, BL, N], F32)             # LSTM2 cell
            nc.vector.memset(c2t[:], 0.0)

            # double-buffered im2col with persistent zero edges
            imt = [st.tile([9, BL, N], BF16, tag=f"im{i}", name=f"im{i}") for i in range(2)]
            for tl in imt:
                nc.vector.memset(tl[:], 0.0)
            # hn staging (written by norm-mul at partitions 0:64, then DMA'd
            # up into xh1 rows 64:128)
            hnl = [st.tile([F1, BL, N], BF16, tag=f"hn{i}", name=f"hn{i}") for i in range(2)]

            # gate tiles (full batch width)
            G0 = gt.tile([128, BL, N], F32, tag="G0")   # L1: f|i
            G1 = gt.tile([128, BL, N], F32, tag="G1")   # L1: o|gc
            G2 = [gt.tile([128, BL, N], F32, tag=f"L2g{g}", name=f"L2g{g}") for g in range(4)]

            def pr(a, p):  # batch-pair slice helper on dim after partitions
                return a[:, 2 * p:2 * p + 2, :]

            # ------------- per-time-step pieces -------------
            def conv1_time(tt):
                """im2col DMA + matmul; writes hconv slot tt%3 (bias folded)."""
                im = imt[tt % 2]
                if tt == 0 or tt == T - 1:
                    # t-edge: some taps fall outside [0,T); clear whole tile
                    # (partition-base of row-range memsets must be 32-aligned)
                    nc.vector.memset(im[:], 0.0)
                for dt in range(3):
                    ts = tt + dt - 1
                    if not (0 <= ts < T):
                        continue
                    for dn in range(3):
                        r = 3 * dt + dn
                        if dn == 0:
                            nc.gpsimd.dma_start(im[r:r + 1, :, 1:N],
                                                xd[:, ts, 0:N - 1])
                        elif dn == 1:
                            nc.gpsimd.dma_start(im[r:r + 1, :, :],
                                                xd[:, ts, :])
                        else:
                            nc.gpsimd.dma_start(im[r:r + 1, :, 0:N - 1],
                                                xd[:, ts, 1:N])
                s = tt % 3
                for p in range(PAIRS):
                    ps = pp.tile([128, 2, N], F32, tag="pair")
                    nc.tensor.matmul(ps[0:F1], w1[:], pr(im, p),
                                     start=True, stop=True)
                    nc.scalar.activation(hconv[:, s, 2 * p:2 * p + 2, 1:N + 1],
                                         ps[0:F1], AF.Identity,
                                         bias=c1b[:], scale=1.0)

            def conv2_l2_hn(t):
                """conv2 + bias, L2-normalize over (N,C) per (b,t), write hn
                staging tile (low partitions) + DMA up into xh1 rows 64:128."""
                g2v = io.tile([F1, BL, N], F32, tag="g2")
                for p in range(PAIRS):
                    ps = pp.tile([128, 2, N], F32, tag="pair")
                    for i9 in range(9):
                        dt, dn = divmod(i9, 3)
                        s = (t + dt - 1) % 3
                        nc.tensor.matmul(
                            ps[0:F1], w2[:, i9, :],
                            hconv[:, s, 2 * p:2 * p + 2, dn:dn + N],
                            start=(i9 == 0), stop=(i9 == 8))
                    nc.scalar.activation(pr(g2v, p), ps[0:F1], AF.Identity,
                                         bias=c2b[:], scale=1.0)
                sq = io.tile([F1, BL, N], F32, tag="sq")
                nc.scalar.activation(sq[:], g2v[:], AF.Square,
                                     bias=zero[0:F1, :], scale=1.0)
                ns = scp.tile([F1, BL], F32, tag="ns")
                nc.vector.tensor_reduce(ns[:], sq[:], AX.X, Op.add)
                psS = pS.tile([128, BL], F32, tag="S")
                nc.tensor.matmul(psS[:], ones[0:F1, :], ns[:],
                                 start=True, stop=True)
                srt = scp.tile([128, BL], F32, tag="srt")
                nc.scalar.activation(srt[:], psS[:], AF.Sqrt,
                                     bias=eps[:], scale=1.0)
                scl = scp.tile([128, BL], F32, tag="scl")
                nc.vector.reciprocal(scl[:], srt[:])
                hn = hnl[t % 2]
                nc.vector.tensor_mul(hn[:], g2v[:],
                                     scl[0:F1, :].to_broadcast((F1, BL, N)))
                # lift hn into the x|h fused tile (rows 64:128, padded cols)
                nc.sync.dma_start(xh1[t % 2][F1:128, :, 1:N + 1], hn[:])

            def lstm1_step(t):
                """LSTM1 step: fused [h1|hn] convs, groups [f|i], [o|gc]."""
                xh = xh1[t % 2]
                for g, gv in enumerate((G0, G1)):
                    gs = slice(128 * g, 128 * (g + 1))
                    for p in range(PAIRS):
                        ps = pp.tile([128, 2, N], F32, tag="pair")
                        for k in range(3):
                            nc.tensor.matmul(ps[:], kxh1[:, k, gs],
                                             xh[:, 2 * p:2 * p + 2, k:k + N],
                                             start=(k == 0), stop=(k == 2))
                        if g == 0:  # [f|i]: both hard-sigmoid
                            nc.scalar.activation(pr(gv, p), ps[:], AF.Relu,
                                                 bias=addc1[:, 0:1], scale=0.2)
                        else:       # [o|gc]
                            nc.scalar.activation(pr(gv, p)[0:64], ps[0:64],
                                                 AF.Relu,
                                                 bias=addc1[0:64, 1:2], scale=0.2)
                            nc.scalar.activation(pr(gv, p)[64:128], ps[64:128],
                                                 AF.Tanh,
                                                 bias=addc1[64:128, 1:2],
                                                 scale=1.0)
                # cell update per pair; write h1 into NEXT xh slot rows 0:64
                xhn = xh1[(t + 1) % 2]
                for p in range(PAIRS):
                    m2h = scp.tile([128, 2, N], F32, tag="m2h")
                    nc.vector.scalar_tensor_tensor(
                        out=m2h[64:128], in0=pr(G0, p)[64:128], scalar=1.0,
                        in1=pr(G1, p)[64:128], op0=Op.min, op1=Op.mult)
                    m2l = scp.tile([64, 2, N], F32, tag="m2l")
                    nc.sync.dma_start(m2l[:], m2h[64:128])
                    m1 = scp.tile([64, 2, N], F32, tag="m1")
                    nc.vector.scalar_tensor_tensor(
                        out=m1[:], in0=pr(G0, p)[0:64], scalar=1.0,
                        in1=pr(c1t, p), op0=Op.min, op1=Op.mult)
                    nc.vector.tensor_add(pr(c1t, p), m1[:], m2l[:])
                    tcv = scp.tile([64, 2, N], F32, tag="tc1")
                    nc.scalar.activation(tcv[:], pr(c1t, p), AF.Tanh,
                                         bias=zero[0:64, :], scale=1.0)
                    nc.vector.scalar_tensor_tensor(
                        out=xhn[0:64, 2 * p:2 * p + 2, 1:N + 1],
                        in0=pr(G1, p)[0:64], scalar=1.0,
                        in1=tcv[:], op0=Op.min, op1=Op.mult)

            def lstm2_step(t):
                """LSTM2 step: kh (h2, early) then kx (h1, late) accumulation;
                groups [i, f, gc, o]."""
                h1t = xh1[(t + 1) % 2]   # rows 0:64 hold h1(t), written this step
                for g in range(4):
                    gs = slice(128 * g, 128 * (g + 1))
                    for p in range(PAIRS):
                        ps = pp.tile([128, 2, N], F32, tag="pair")
                        for k in range(3):
                            nc.tensor.matmul(ps[:], kh2[:, k, gs],
                                             h2p[:, 2 * p:2 * p + 2, k:k + N],
                                             start=(k == 0), stop=False)
                        for k in range(3):
                            nc.tensor.matmul(ps[:], kx2[:, k, gs],
                                             h1t[0:F1, 2 * p:2 * p + 2, k:k + N],
                                             start=False, stop=(k == 2))
                        gv = G2[g]
                        if g == 2:  # gc: tanh
                            nc.scalar.activation(pr(gv, p), ps[:], AF.Tanh,
                                                 bias=addc2[:, g:g + 1],
                                                 scale=1.0)
                        else:       # i/f/o: hard-sigmoid
                            nc.scalar.activation(pr(gv, p), ps[:], AF.Relu,
                                                 bias=addc2[:, g:g + 1],
                                                 scale=0.2)
                # cell update per pair
                for p in range(PAIRS):
                    m1 = scp.tile([128, 2, N], F32, tag="m1b")
                    nc.vector.scalar_tensor_tensor(
                        out=m1[:], in0=pr(G2[1], p), scalar=1.0,
                        in1=pr(c2t, p), op0=Op.min, op1=Op.mult)
                    m2 = scp.tile([128, 2, N], F32, tag="m2b")
                    nc.vector.scalar_tensor_tensor(
                        out=m2[:], in0=pr(G2[0], p), scalar=1.0,
                        in1=pr(G2[2], p), op0=Op.min, op1=Op.mult)
                    nc.vector.tensor_add(pr(c2t, p), m1[:], m2[:])
                    tcv = scp.tile([128, 2, N], F32, tag="tc2")
                    nc.scalar.activation(tcv[:], pr(c2t, p), AF.Tanh,
                                         bias=zero[:], scale=1.0)
                    nc.vector.scalar_tensor_tensor(
                        out=h2p[:, 2 * p:2 * p + 2, 1:N + 1],
                        in0=pr(G2[3], p), scalar=1.0,
                        in1=tcv[:], op0=Op.min, op1=Op.mult)

            # ---------------- main time loop ----------------
            conv1_time(0)
            for t in range(T):
                if t + 1 < T:
                    conv1_time(t + 1)
                else:
                    nc.vector.memset(hconv[:, (t + 1) % 3, :, :], 0.0)
                conv2_l2_hn(t)
                lstm1_step(t)
                lstm2_step(t)

            # ---------------- final L2 norm + FC ----------------
            sq2 = io.tile([F2, BL, N], F32, tag="sq2")
            nc.scalar.activation(sq2[:], h2p[:, :, 1:N + 1], AF.Square,
                                 bias=zero[:], scale=1.0)
            ns2 = scp.tile([F2, BL], F32, tag="ns")
            nc.vector.tensor_reduce(ns2[:], sq2[:], AX.X, Op.add)
            psS2 = pS.tile([128, BL], F32, tag="S")
            nc.tensor.matmul(psS2[:], ones[:], ns2[:], start=True, stop=True)
            srt2 = scp.tile([128, BL], F32, tag="srt")
            nc.scalar.activation(srt2[:], psS2[:], AF.Sqrt, bias=eps[:], scale=1.0)
            scl2 = scp.tile([128, BL], F32, tag="scl")
            nc.vector.reciprocal(scl2[:], srt2[:])
            ysb = io.tile([1, BL, N], F32, tag="y")
            for p in range(PAIRS):
                psY = pp.tile([128, 2, N], F32, tag="pair")
                nc.tensor.matmul(psY[0:1], fcw[:], h2p[:, 2 * p:2 * p + 2, 1:N + 1],
                                 start=True, stop=True)
                nc.vector.tensor_mul(pr(ysb, p), psY[0:1],
                                     scl2[0:1, 2 * p:2 * p + 2]
                                     .to_broadcast((1, 2, N)))
                nc.vector.tensor_scalar_add(pr(ysb, p), pr(ysb, p),
                                            fcb[0:1, 0:1])
            nc.sync.dma_start(y_d.rearrange("(u b) n -> u b n", u=1), ysb[:])

    from concourse import mybir as _mybir
    _legalize_waits(nc, _mybir)
    return nc


def kernel(**inputs):
    from concourse.bass_utils import run_bass_kernel_spmd

    if "nc" not in _CACHE:
        _CACHE["nc"] = _build_nc()
    nc = _CACHE["nc"]

    x = np.ascontiguousarray(np.asarray(inputs["x"], dtype=np.float32))
    shared = {k: np.ascontiguousarray(np.asarray(v, dtype=np.float32))
              for k, v in inputs.items() if k != "x"}
    shared["fc_w"] = shared["fc_w"].reshape(F2, 1)
    in_maps = []
    for c in range(NCORES):
        m = dict(shared)
        m["x"] = x[c * BL:(c + 1) * BL]
        in_maps.append(m)

    res = run_bass_kernel_spmd(nc, in_maps, core_ids=list(range(NCORES)))
    y = np.concatenate([res.results[c]["y"] for c in range(NCORES)], axis=0)
    return y.reshape(B_FULL, 1, N, 1).astype(np.float32)
